# revision 17
# baseline (speedup 1.0000x reference)
"""Trainium2 Bass kernel for nn_MinDistanceConvLayer2.

out[b,c,i,j] = max_{x,y} ( -sqrt((x-i)^2 + (y-j)^2) - f[b,c,x,y] )

Algorithm: the candidate q=(i,j) itself gives value -f[i,j], so the argmax
(x,y) for output pixel p satisfies D(p,q) <= f[p] - f[q] <= max(f) - min(f);
the global max-plus product with the 9216x9216 distance matrix collapses to a
local max-plus convolution with a small tap window.  Taps are pruned exactly,
per offset delta: delta is needed iff exists p: f[p] - f[p+delta] > T[delta]
(otherwise it can never strictly beat the center tap).

Sharding: output rows split into 8 blocks of 12 (one per core).  Each core
receives a dy-replicated transposed slab of g = -f covering its rows + halo:
    slab[j, dy'*SLAB_F + i'] = gpad[12*core + i', j + dy']
(j on partitions; the dy replication happens on host because SBUF compute
access patterns must start at 32-aligned partitions, so dy cannot be a
partition offset).

Device program (raw bass, identical on all cores; data differs):
    1. one DMA in: [slab | merged-constants] -> SBUF
    2. for each |dy| pair: one tensor_tensor(max) folds the +dy and -dy
       tap columns (equal distance constants) into a packed tile
       mpack[j, i*TM + t]; dy=0 columns are tensor_copy'd
    3. one tensor_tensor(subtract) of the constants (stride-0 broadcast
       along i) over the packed tile
    4. one tensor_reduce(max) over the innermost tap axis -> res[j, i]
    5. one DMA out
Host stitches the 8 [96,12] results into [96,96].  All arithmetic is fp32
and exact (the window provably contains the argmax).
"""

import numpy as np

H = W = 96
NC = 8
BLK = H // NC  # 12 output rows per core

_cache: dict = {}


def _tap_plan(f: np.ndarray):
    """Exact tap-set computation + merged |dy| grouping.

    Returns (plan, dymax, dxmax, TM, c2) where plan is a list of
    (ady, dx0, K, col0) merged groups: for ady>0 the group covers taps
    (dx, +ady) and (dx, -ady) for dx in [dx0, dx0+K); for ady==0 it covers
    (dx, 0).  col0 is the group's starting column in the packed tile; TM the
    total packed columns; c2[t] the fp32 distance constant per column.
    """
    fmin = float(f.min())
    fmax = float(f.max())
    span = fmax - fmin
    Rmax = max(1, int(np.ceil(span)))
    kept = {(0, 0)}
    for dx in range(-Rmax, Rmax + 1):
        for dy in range(-Rmax, Rmax + 1):
            if dx == 0 and dy == 0:
                continue
            T = float(np.hypot(dx, dy))
            if T >= span:
                continue
            x0, x1 = max(0, -dx), min(H, H - dx)
            y0, y1 = max(0, -dy), min(W, W - dy)
            if x0 >= x1 or y0 >= y1:
                continue
            diff = f[x0:x1, y0:y1] - f[x0 + dx:x1 + dx, y0 + dy:y1 + dy]
            if float(diff.max()) > T:
                kept.add((dx, dy))
    dymax = max(abs(dy) for _, dy in kept)
    dxmax = max(abs(dx) for dx, _ in kept)
    plan = []
    c2 = []
    col0 = 0
    for ady in range(dymax + 1):
        dxs = [dx for dx, dy in kept if abs(dy) == ady]
        if not dxs:
            continue
        dx0, dx1 = min(dxs), max(dxs)
        K = dx1 - dx0 + 1
        plan.append((ady, dx0, K, col0))
        for dx in range(dx0, dx1 + 1):
            c2.append(np.float32(np.hypot(dx, ady)))
        col0 += K
    return plan, dymax, dxmax, col0, np.array(c2, dtype=np.float32)


def _split_waits(nc, limit=1):
    """This walrus build allows only `limit` sync-wait per instruction;
    hoist excess waits onto preceding same-engine NoOps."""
    import concourse.mybir as mybir

    for bb in nc.m.functions[0].blocks:
        i = 0
        while i < len(bb.instructions):
            ins = bb.instructions[i]
            si = getattr(ins, 'sync_info', None)
            if si is not None and len(si.on_wait) > limit:
                waits = list(si.on_wait)
                extra, keep = waits[:-limit], waits[-limit:]
                pos = i
                for j in range(0, len(extra), limit):
                    chunk = extra[j:j + limit]
                    nop = mybir.InstNoOp(name=f"W-{ins.name}-{j}", ins=[],
                                         outs=[])
                    nop.engine = ins.engine
                    nop.sync_info = mybir.SyncInfo(on_wait=chunk, on_update=[])
                    bb.instructions.insert(pos, nop)
                    pos += 1
                si.on_wait[:] = keep
                i = pos
            i += 1
    return nc


def _build_program(plan, TM, dymax, dxmax, gp_folds=0, act_copy=True):
    import concourse.bass as bass
    import concourse.mybir as mybir
    from concourse.bass_types import AP

    f32 = mybir.dt.float32
    NDY = 2 * dymax + 1
    SLAB_F = BLK + 2 * dxmax
    SLAB_W = NDY * SLAB_F

    nc = bass.Bass()
    comb_d = nc.declare_dram_parameter("comb", [H, SLAB_W + TM], f32,
                                       isOutput=False)
    out_d = nc.declare_dram_parameter("res", [H, BLK], f32, isOutput=True)

    folds = [g for g in plan if g[0] != 0]
    n_gp = min(gp_folds, len(folds))
    dve_folds = folds[:len(folds) - n_gp]
    gpsimd_folds = folds[len(folds) - n_gp:]
    dy0 = [g for g in plan if g[0] == 0]
    # gpsimd subtracts exactly the columns it folded (no cross-engine dep);
    # DVE subtracts the rest.
    subcut = gpsimd_folds[0][3] if gpsimd_folds else TM
    n_act = len(dy0) if act_copy else 0

    with (
        nc.sbuf_tensor([H, SLAB_W + TM], f32) as comb_t,
        nc.sbuf_tensor([H, BLK * TM], f32) as mpack,
        nc.sbuf_tensor([H, BLK], f32) as res_t,
        nc.semaphore("dma_sem") as dma_sem,
        nc.semaphore("dve_sem") as dve_sem,
        nc.semaphore("gp_sem") as gp_sem,
        nc.semaphore("act_sem") as act_sem,
        nc.Block() as block,
    ):
        s_ap = comb_t[:]
        srow = s_ap.ap[0][0]
        p_ap = mpack[:]
        prow = p_ap.ap[0][0]

        def slab_ap(dy, dx0, K):
            off = (dy + dymax) * SLAB_F + (dx0 + dxmax)
            return AP(s_ap.tensor, off, [[srow, H], [1, BLK], [1, K]])

        def pk_ap(col0, K):
            return AP(p_ap.tensor, col0, [[prow, H], [TM, BLK], [1, K]])

        @block.sync
        def _(sync):
            sync.dma_start(out=comb_t[:], in_=comb_d[:]).then_inc(dma_sem, 16)
            sync.wait_ge(dve_sem, 1)
            sync.dma_start(out=out_d[:], in_=res_t[:]).then_inc(dma_sem, 16)

        @block.gpsimd
        def _(gpsimd):
            if gpsimd_folds:
                gpsimd.wait_ge(dma_sem, 16)
                for (ady, dx0, K, col0) in gpsimd_folds:
                    nc.gpsimd.tensor_tensor(
                        out=pk_ap(col0, K), in0=slab_ap(ady, dx0, K),
                        in1=slab_ap(-ady, dx0, K),
                        op=mybir.AluOpType.max)
                # subtract constants on the columns this engine folded
                # (program order covers the dependency)
                k2 = TM - subcut
                tt = AP(p_ap.tensor, subcut,
                        [[prow, H], [TM, BLK], [1, k2]])
                cb = AP(s_ap.tensor, SLAB_W + subcut,
                        [[srow, H], [0, BLK], [1, k2]])
                nc.gpsimd.tensor_tensor(
                    out=tt, in0=tt, in1=cb,
                    op=mybir.AluOpType.subtract).then_inc(gp_sem, 1)
            # End-of-kernel semaphore hygiene: sems are NOT cleared on
            # allocation, and the runtime may keep the NEFF loaded across
            # invocations — a re-execution with dirty semaphores races
            # ahead of the DMAs and crashes the core.  Wait for both DMAs
            # (in 16 + out 16) then reset everything we touched.
            gpsimd.wait_ge(dma_sem, 32)
            for s in (dma_sem, dve_sem, act_sem, gp_sem):
                gpsimd.sem_clear(s)

        if n_act:
            @block.scalar
            def _(scalar):
                scalar.wait_ge(dma_sem, 16)
                for (ady, dx0, K, col0) in dy0:
                    nc.scalar.copy(pk_ap(col0, K),
                                   slab_ap(0, dx0, K)).then_inc(act_sem, 1)

        @block.vector
        def _(vector):
            vector.wait_ge(dma_sem, 16)
            if not n_act:
                for (ady, dx0, K, col0) in dy0:
                    nc.vector.tensor_copy(pk_ap(col0, K), slab_ap(0, dx0, K))
            for (ady, dx0, K, col0) in dve_folds:
                nc.vector.tensor_tensor(out=pk_ap(col0, K),
                                        in0=slab_ap(ady, dx0, K),
                                        in1=slab_ap(-ady, dx0, K),
                                        op=mybir.AluOpType.max)
            if n_act:
                vector.wait_ge(act_sem, n_act)
            # mpack -= c  (constants broadcast along i via stride-0 dim)
            tt = AP(p_ap.tensor, 0, [[prow, H], [TM, BLK], [1, subcut]])
            c_b = AP(s_ap.tensor, SLAB_W, [[srow, H], [0, BLK], [1, subcut]])
            nc.vector.tensor_tensor(out=tt, in0=tt, in1=c_b,
                                    op=mybir.AluOpType.subtract)
            if gpsimd_folds:
                vector.wait_ge(gp_sem, 1)
            red_in = AP(p_ap.tensor, 0, [[prow, H], [TM, BLK], [1, TM]])
            nc.vector.tensor_reduce(
                res_t[:], red_in, axis=mybir.AxisListType.X,
                op=mybir.AluOpType.max).then_inc(dve_sem, 1)

    return _split_waits(nc)


def _get_compiled(f: np.ndarray):
    plan, dymax, dxmax, TM, c2 = _tap_plan(f)
    key = tuple(plan)
    if key not in _cache:
        nc = _build_program(plan, TM, dymax, dxmax)
        _cache[key] = (nc, plan, dymax, dxmax, TM, c2)
    return _cache[key]


def _prepare(f: np.ndarray):
    """Returns (nc, in_maps) for the given 96x96 feature map."""
    nc, plan, dymax, dxmax, TM, c2 = _get_compiled(f)

    g = -f
    NDY = 2 * dymax + 1
    SLAB_F = BLK + 2 * dxmax
    gpad = np.full((H + 2 * dxmax, W + 2 * dymax), -1e30, dtype=np.float32)
    gpad[dxmax:dxmax + H, dymax:dymax + W] = g
    cvec = np.tile(c2[None, :], (H, 1))
    in_maps = []
    for c in range(NC):
        sub = gpad[BLK * c: BLK * c + SLAB_F, :]          # [SLAB_F, W+2dymax]
        swv = np.lib.stride_tricks.sliding_window_view(sub, W, axis=1)
        # swv[i', dy', j] = sub[i', dy' + j]  -> want slab2[j, dy', i']
        slab2 = swv.transpose(2, 1, 0).reshape(H, NDY * SLAB_F)
        comb = np.concatenate([slab2, cvec], axis=1)
        in_maps.append({"comb": np.ascontiguousarray(comb)})
    return nc, in_maps


def kernel(feature_map: np.ndarray) -> np.ndarray:
    from concourse.bass_utils import run_bass_kernel_spmd

    fm = np.asarray(feature_map, dtype=np.float32)
    B, C, _, _ = fm.shape
    f = fm[0, 0]
    nc, in_maps = _prepare(f)

    results = run_bass_kernel_spmd(nc, in_maps, list(range(NC))).results

    out = np.empty((H, W), dtype=np.float32)
    for c in range(NC):
        out[BLK * c: BLK * (c + 1), :] = results[c]["res"].T
    return out.reshape(B, C, H, W)
